# revision 5
# baseline (speedup 1.0000x reference)
"""Trainium2 Bass kernel for nn_ConvTran (conv stem + eRPE transformer + GAP).

v2 redesign: data parallel (2 items/core, 8 cores). Matmuls in bf16 except the
q/k/score path (fp32 for softmax accuracy). ACT (scalar engine) is the
bottleneck: gelu on the conv stem + softmax exp. All activations are batched
into 2048-wide calls reading multi-bank PSUM tiles. LayerNorm rstd uses
exp(-0.5*ln(v+eps)) to stay inside the natural_log_exp table set (no ACT table
thrash). The softmax division is folded away via LN scale-invariance:
LN(pv/D + bv) == LN(pv + D*bv).
"""

import os
import numpy as np
import ml_dtypes

KDBG = bool(os.environ.get("KDBG"))
KDBG_G = int(os.environ.get("KDBG_G", "0"))

B, S, C_IN, E, H, DFF = 16, 1024, 4, 128, 8, 512
C1 = E * 4          # 512
DH = E // H         # 16
EPS = 1e-5
SCALE = float(E) ** -0.5
N_CORES = 8
NB = B // N_CORES   # 2 items per core
NG = 2              # head groups of 4
F32 = np.float32
BF16 = ml_dtypes.bfloat16


class _Pack:
    """Column-packed [128, N] constant store."""

    def __init__(self, npdtype):
        self.cols = []
        self.index = {}
        self.n = 0
        self.npdtype = npdtype

    def add(self, name, arr2d):
        a = np.zeros((128, arr2d.shape[1]), self.npdtype)
        a[:arr2d.shape[0]] = arr2d.astype(self.npdtype)
        self.index[name] = (self.n, arr2d.shape[1])
        self.cols.append(a)
        self.n += arr2d.shape[1]

    def finalize(self):
        return np.ascontiguousarray(np.concatenate(self.cols, axis=1))


def _host_prep(inp):
    f = lambda a: np.asarray(a, dtype=F32)
    fpk = _Pack(F32)
    bpk = _Pack(BF16)

    # ---- conv1 (+bn1 folded): packed 4 row-groups, bias as 9th K-row ----
    w1 = f(inp["conv1_w"])[:, 0, 0, :]                # [C1, 8]
    sA = f(inp["bn1_g"]) / np.sqrt(f(inp["bn1_v"]) + EPS)
    bA = (f(inp["conv1_b"]) - f(inp["bn1_m"])) * sA + f(inp["bn1_b"])
    w1s = w1 * sA[:, None]                            # scale folded
    w1cP = np.zeros((128, 128), F32)
    for cc in range(4):
        sl = slice(cc * 128, (cc + 1) * 128)
        w1cP[32 * cc:32 * cc + 8, :] = w1s[sl, :].T
        w1cP[32 * cc + 8, :] = bA[sl]
    bpk.add("w1cP", w1cP)

    # ---- conv2 (+bn2 via ACT scale/bias) ----
    w2 = f(inp["conv2_w"])[:, :, :, 0]                # [E, C1, 4]
    w2cT = np.zeros((128, 16, 128), F32)
    for r in range(4):
        for cc in range(4):
            w2cT[:, r * 4 + cc, :] = w2[:, cc * 128:(cc + 1) * 128, r].T
    bpk.add("w2cT", w2cT.reshape(128, 16 * 128))
    sB = f(inp["bn2_g"]) / np.sqrt(f(inp["bn2_v"]) + EPS)
    fpk.add("scaleB", sB[:, None])
    fpk.add("biasB", ((f(inp["conv2_b"]) - f(inp["bn2_m"])) * sB
                      + f(inp["bn2_b"]))[:, None])

    # ---- tAPE positional encoding, transposed [E, S] ----
    pos = np.arange(S, dtype=np.float64)[:, None]
    div = np.exp(np.arange(0, E, 2, dtype=np.float64) * (-np.log(10000.0) / E))
    ang = pos * div * (E / S)
    pe = np.zeros((S, E), np.float64)
    pe[:, 0::2] = np.sin(ang)
    pe[:, 1::2] = np.cos(ang)
    fpk.add("peT", pe.astype(F32).T)

    # ---- q/k weights, padded head layout (fp32 path) ----
    def pad_qk(w):
        w = f(w)
        wt = np.zeros((128, NG * 128), F32)
        for g in range(NG):
            for c in range(4):
                h = 4 * g + c
                wt[:, g * 128 + 32 * c:g * 128 + 32 * c + DH] = \
                    w[h * DH:(h + 1) * DH, :].T
        return wt
    fpk.add("wqT", pad_qk(inp["wq"]))
    fpk.add("wkT", pad_qk(inp["wk"]))
    fpk.add("wvT", f(inp["wv"]).T)

    bpk.add("ffw1T", f(inp["ff_w1"]).T)
    fpk.add("ffb1", f(inp["ff_b1"]).reshape(4, 128).T)
    bpk.add("ffw2T", f(inp["ff_w2"]).T.reshape(4, 128, 128)
            .transpose(1, 0, 2).reshape(128, 512))
    fpk.add("ffb2", f(inp["ff_b2"])[:, None])

    fpk.add("eps", np.full((128, 1), EPS, F32))
    fpk.add("invS", np.full((128, 1), 1.0 / S, F32))
    fpk.add("onesRow", np.ones((1, 128), F32))
    fpk.add("identF", np.eye(128, dtype=F32))
    bpk.add("identB", np.eye(128, dtype=F32))
    # gather: rr[c, :] = osb[32*c+16, :]
    g4 = np.zeros((128, 4), F32)
    for c in range(4):
        g4[32 * c + 16, c] = 1.0
    fpk.add("gather4", g4)
    # broadcast: rsb2[q, :] = rinv[q//32, :]
    b4 = np.zeros((4, 128), F32)
    for q in range(128):
        b4[q // 32, q] = 1.0
    fpk.add("bcast4", b4)
    bpk.add("onesb", np.ones((128, 1), F32))
    fpk.add("onesf", np.ones((128, 1), F32))

    lnG = np.stack([f(inp["ln_attn_g"]), f(inp["ln1_g"]), f(inp["ln2_g"])])
    lnB = np.stack([f(inp["ln_attn_b"]), f(inp["ln1_b"]), f(inp["ln2_b"])])
    ln_identity = bool(np.allclose(lnG, 1.0) and np.allclose(lnB, 0.0))
    fpk.add("lnG", np.broadcast_to(lnG.reshape(1, 3 * 128), (128, 384)).copy())
    fpk.add("lnB", np.broadcast_to(lnB.reshape(1, 3 * 128), (128, 384)).copy())

    d = {"fpack": fpk.finalize(), "bpack": bpk.finalize()}

    # rel_bias diagonal store (bf16): T[jj, h, c] = rel[127 + c - jj, h]
    rel = f(inp["rel_bias"])                          # [2047, 8]
    jj = np.arange(128)[:, None]
    cidx = np.arange(1920)[None, :]
    ts = rel[127 + cidx - jj, :]                      # [128, 1920, 8]
    d["tstore"] = np.ascontiguousarray(
        ts.transpose(0, 2, 1).astype(BF16))           # [128, 8, 1920]
    return d, fpk.index, bpk.index, ln_identity


def _build_bass(fidx, bidx, nf, nb, ln_identity):
    import concourse.bass as bass
    import concourse.bacc as bacc
    import concourse.tile as tile
    import concourse.mybir as mybir
    import contextlib

    dt = mybir.dt
    AF = mybir.ActivationFunctionType
    ALU = mybir.AluOpType

    nc = bacc.Bacc("TRN2")

    xin = nc.dram_tensor("rhs_rep", [NB, 128, 4096], dt.bfloat16,
                         kind="ExternalInput")
    fpk_dr = nc.dram_tensor("fpack", [128, nf], dt.float32,
                            kind="ExternalInput")
    bpk_dr = nc.dram_tensor("bpack", [128, nb], dt.bfloat16,
                            kind="ExternalInput")
    ts_dr = nc.dram_tensor("tstore", [128, H, 1920], dt.bfloat16,
                           kind="ExternalInput")
    yout = nc.dram_tensor("y", [NB, E], dt.float32, kind="ExternalOutput")
    dbg = {}
    if KDBG:
        for nm, shp, ddt in [("xsT", [128, S], dt.float32),
                             ("xpT", [128, S], dt.float32),
                             ("qT0", [128, S], dt.float32),
                             ("kT0", [128, S], dt.float32),
                             ("ut00", [128, 2048], dt.bfloat16),
                             ("osb00", [128, 512], dt.float32),
                             ("osb2_00", [128, 512], dt.float32),
                             ("oatt0", [128, 1024], dt.bfloat16),
                             ("att0", [128, 1024], dt.float32),
                             ("ffT", [128, S], dt.float32),
                             ("h1d", [128, 4096], dt.bfloat16)]:
            dbg[nm] = nc.dram_tensor("dbg_" + nm, shp, ddt,
                                     kind="ExternalOutput")

    with tile.TileContext(nc) as tc:
        ctx = contextlib.ExitStack()
        with ctx:
            consts = ctx.enter_context(tc.tile_pool(name="consts", bufs=1))
            bpk = consts.tile([128, nb], dt.bfloat16, tag="bpack")
            nc.sync.dma_start(out=bpk[:, 0:128], in_=bpk_dr[:, 0:128])
            nc.sync.dma_start(out=bpk[:, 128:], in_=bpk_dr[:, 128:])
            rhsr = [consts.tile([128, 4096], dt.bfloat16, tag=f"rhsr{b}",
                                name=f"rhsr{b}") for b in range(NB)]
            nc.sync.dma_start(out=rhsr[0], in_=xin[0])
            fpk = consts.tile([128, nf], dt.float32, tag="fpack")
            nc.sync.dma_start(out=fpk, in_=fpk_dr[:])
            nc.sync.dma_start(out=rhsr[1], in_=xin[1])
            ts_sb = consts.tile([128, H, 1920], dt.bfloat16, tag="tstore")
            nc.sync.dma_start(out=ts_sb, in_=ts_dr[:])

            def FC(name, rows=128):
                o, w = fidx[name]
                return fpk[0:rows, o:o + w]

            def BC(name, rows=128):
                o, w = bidx[name]
                return bpk[0:rows, o:o + w]

            w1cP = BC("w1cP")
            w2cT = BC("w2cT").rearrange("p (k e) -> p k e", k=16)
            scaleB, biasB = FC("scaleB"), FC("biasB")
            peT = FC("peT")
            wqT = FC("wqT").rearrange("p (g e) -> p g e", g=NG)
            wkT = FC("wkT").rearrange("p (g e) -> p g e", g=NG)
            wvT = FC("wvT")
            ffw1T = BC("ffw1T")
            ffb1 = FC("ffb1")
            ffw2T = BC("ffw2T").rearrange("p (k e) -> p k e", k=4)
            ffb2 = FC("ffb2")
            eps_ap = FC("eps")
            invS = FC("invS")
            onesRow = FC("onesRow", rows=1)
            identF = FC("identF")
            identB = BC("identB")
            gather4 = FC("gather4")
            bcast4 = FC("bcast4", rows=4)
            onesb = BC("onesb")
            onesf = FC("onesf")
            lnG = FC("lnG").rearrange("p (k e) -> p k e", k=3)
            lnB = FC("lnB").rearrange("p (k e) -> p k e", k=3)

            pers = ctx.enter_context(tc.tile_pool(name="pers", bufs=1))
            work = ctx.enter_context(tc.tile_pool(name="work", bufs=1))
            pp = ctx.enter_context(tc.tile_pool(name="pp", bufs=1,
                                                space="PSUM"))

            def T2048():
                return pp.tile([128, 2048], dt.float32, tag="T2048",
                               name="T2048")

            def TA():
                return pp.tile([128, 512], dt.float32, tag="TA", name="TA")

            def TB():
                return pp.tile([128, 512], dt.float32, tag="TB", name="TB")

            def TR():
                return pp.tile([128, 512], dt.float32, tag="TR", name="TR")

            def TO():
                return pp.tile([128, 512], dt.float32, tag="TO", name="TO")

            # persistent per-item tiles
            xpTf = [pers.tile([128, S], dt.float32, tag=f"xpTf{b}",
                              name=f"xpTf{b}") for b in range(NB)]
            xsrc = [pers.tile([128, 8, 128], dt.bfloat16, tag=f"xsrc{b}",
                              name=f"xsrc{b}") for b in range(NB)]
            qT = [[pers.tile([128, S], dt.bfloat16, tag=f"qT{b}{g}",
                             name=f"qT{b}{g}") for g in range(NG)]
                  for b in range(NB)]
            kT = [[pers.tile([128, S], dt.bfloat16, tag=f"kT{b}{g}",
                             name=f"kT{b}{g}") for g in range(NG)]
                  for b in range(NB)]
            VOall = [pers.tile([128, 8, 8, 32], dt.bfloat16, tag=f"VO{b}",
                               name=f"VO{b}") for b in range(NB)]
            oatt = [pers.tile([128, 8, 8, 16], dt.bfloat16, tag=f"oatt{b}",
                              name=f"oatt{b}") for b in range(NB)]

            # ================= conv stem =================
            def stem(b):
                rhs_b = rhsr[b]
                h1 = work.tile([128, 4, 4, 1024], dt.bfloat16, tag="h1",
                               name="h1", bufs=1)
                for n in range(8):          # (r, sh) chunk of 512 positions
                    r, sh = n // 2, n % 2
                    ps = T2048()
                    psr = ps.rearrange("p (c i) -> p c i", c=4)
                    for cc in range(4):
                        nc.tensor.matmul(
                            psr[:, cc, :],
                            lhsT=w1cP[32 * cc:32 * cc + 9, :],
                            rhs=rhs_b[32 * cc:32 * cc + 9,
                                      n * 512:(n + 1) * 512],
                            start=True, stop=True,
                            tile_position=(32 * cc, 0))
                    nc.scalar.activation(
                        h1[:, :, r, sh * 512:(sh + 1) * 512], psr, AF.Gelu)
                if KDBG and b == 0:
                    h1v = h1.rearrange("p a b s -> p (a b s)")
                    nc.sync.dma_start(out=dbg["h1d"][:], in_=h1v[:, 0:4096])
                xsT = work.tile([128, S], dt.float32, tag="xsT", name="xsT",
                                bufs=2)
                for sh in range(2):
                    ps = TA() if sh == 0 else TB()
                    for k in range(16):
                        r, cc = k // 4, k % 4
                        nc.tensor.matmul(
                            ps, lhsT=w2cT[:, k, :],
                            rhs=h1[:, cc, r, sh * 512:(sh + 1) * 512],
                            start=(k == 0), stop=(k == 15))
                    nc.scalar.activation(
                        xsT[:, sh * 512:(sh + 1) * 512], ps, AF.Gelu,
                        bias=biasB, scale=scaleB)
                if KDBG and b == 0:
                    nc.sync.dma_start(out=dbg["xsT"][:], in_=xsT)
                nc.vector.tensor_tensor(xpTf[b], xsT, peT, op=ALU.add)
                if KDBG and b == 0:
                    nc.sync.dma_start(out=dbg["xpT"][:], in_=xpTf[b])
                for half in range(2):
                    ps = TO()
                    psr = ps.rearrange("p (k m) -> p k m", k=4)
                    for k in range(4):
                        sc = half * 4 + k
                        nc.tensor.transpose(
                            psr[:, k, :], xsT[:, sc * 128:(sc + 1) * 128],
                            identF)
                    nc.vector.tensor_copy(
                        out=xsrc[b][:, half * 4:(half + 1) * 4, :], in_=psr)

            # ================= qkv =================
            def qkv(b):
                for g in range(NG):
                    for sh in range(2):
                        ps = TR()
                        nc.tensor.matmul(ps, lhsT=wqT[:, g, :],
                                         rhs=xpTf[b][:, sh * 512:(sh + 1) * 512],
                                         start=True, stop=True)
                        nc.vector.tensor_copy(
                            out=qT[b][g][:, sh * 512:(sh + 1) * 512], in_=ps)
                        ps2 = TR()
                        nc.tensor.matmul(ps2, lhsT=wkT[:, g, :],
                                         rhs=xpTf[b][:, sh * 512:(sh + 1) * 512],
                                         start=True, stop=True)
                        nc.vector.tensor_copy(
                            out=kT[b][g][:, sh * 512:(sh + 1) * 512], in_=ps2)
                    if KDBG and b == 0 and g == KDBG_G:
                        nc.sync.dma_start(out=dbg["qT0"][:], in_=qT[b][g])
                        nc.sync.dma_start(out=dbg["kT0"][:], in_=kT[b][g])
                vo = VOall[b]
                vof = vo.rearrange("p a b c -> p (a b c)")
                nc.vector.memset(vof, 0.0)
                for half in range(2):
                    ps = TO()
                    psr = ps.rearrange("p (k m) -> p k m", k=4)
                    for k in range(4):
                        jc = half * 4 + k
                        nc.tensor.matmul(
                            psr[:, k, :],
                            lhsT=xpTf[b][:, jc * 128:(jc + 1) * 128],
                            rhs=wvT, start=True, stop=True)
                    nc.vector.tensor_copy(
                        out=vo[:, half * 4:(half + 1) * 4, :, 0:16],
                        in_=psr.rearrange("p k (h d) -> p k h d", h=8))
                nc.vector.memset(vo[:, :, :, 16:17], 1.0)

            # ================= attention =================
            def attn(b, g, ih):
                i0 = ih * 512
                pv = TA()
                bv = TB()

                def emit_scores(s4v, jc, crng):
                    for c in crng:
                        nc.tensor.matmul(
                            s4v[:, c, :],
                            lhsT=kT[b][g][32 * c:32 * c + DH,
                                          jc * 128:(jc + 1) * 128],
                            rhs=qT[b][g][32 * c:32 * c + DH, i0:i0 + 512],
                            start=True, stop=True,
                            tile_position=(32 * c, 0))

                s4 = T2048().rearrange("p (c i) -> p c i", c=4)
                emit_scores(s4, 0, range(4))
                for jc in range(8):
                    ut = work.tile([128, 4, 512], dt.bfloat16, tag="ut",
                                   name="ut", bufs=5)
                    nc.scalar.activation(ut[:, 0:2, :], s4[:, 0:2, :],
                                         AF.Exp, scale=SCALE)
                    if jc < 7:
                        s4n = T2048().rearrange("p (c i) -> p c i", c=4)
                        emit_scores(s4n, jc + 1, range(0, 2))
                    nc.scalar.activation(ut[:, 2:4, :], s4[:, 2:4, :],
                                         AF.Exp, scale=SCALE)
                    if jc < 7:
                        emit_scores(s4n, jc + 1, range(2, 4))
                    if KDBG and b == 0 and g == KDBG_G and ih == 0 and jc == 0:
                        utf = ut.rearrange("p c i -> p (c i)")
                        nc.sync.dma_start(out=dbg["ut00"][:], in_=utf)
                    for c in range(4):
                        nc.tensor.matmul(
                            pv[32 * c:32 * c + 32, :],
                            lhsT=VOall[b][:, jc, 4 * g + c, 0:32],
                            rhs=ut[:, c, :],
                            start=(jc == 0), stop=(jc == 7),
                            skip_group_check=True,
                            tile_position=(0, 32 * c))
                        nc.tensor.matmul(
                            bv[32 * c:32 * c + 32, :],
                            lhsT=VOall[b][:, jc, 4 * g + c, 0:32],
                            rhs=ts_sb[:, 4 * g + c,
                                      896 - 128 * jc + i0:
                                      896 - 128 * jc + i0 + 512],
                            start=(jc == 0), stop=(jc == 7),
                            skip_group_check=True,
                            tile_position=(0, 32 * c))
                    if jc < 7:
                        s4 = s4n
                osb = work.tile([128, 512], dt.float32, tag="osb",
                                name="osb", bufs=2)
                nc.vector.tensor_copy(out=osb, in_=pv)
                rr = TO()
                nc.tensor.matmul(rr[0:4, :], lhsT=gather4, rhs=osb,
                                 start=True, stop=True)
                rinv = work.tile([4, 512], dt.float32, tag="rinv",
                                 name="rinv", bufs=2)
                nc.vector.reciprocal_approx_fast(out=rinv, in_=rr[0:4, :])
                rsb2 = TO()
                nc.tensor.matmul(rsb2, lhsT=bcast4, rhs=rinv, start=True,
                                 stop=True)
                osbn = work.tile([128, 512], dt.bfloat16, tag="osbn",
                                 name="osbn", bufs=2)
                nc.vector.tensor_tensor(osbn, osb, rsb2, op=ALU.mult)
                osb2 = work.tile([128, 512], dt.float32, tag="osb2",
                                 name="osb2", bufs=1)
                nc.vector.tensor_tensor(osb2, osbn, bv, op=ALU.add)
                if KDBG and b == 0 and g == KDBG_G and ih == 0:
                    nc.sync.dma_start(out=dbg["osb00"][:], in_=osb)
                    nc.sync.dma_start(out=dbg["osb2_00"][:], in_=osb2)
                oT = TO()
                oTr = oT.rearrange("p (ic c d) -> p ic c d", ic=4, c=4)
                for ic in range(4):
                    nc.tensor.transpose(
                        oTr.rearrange("p ic c d -> p ic (c d)")[:, ic, :],
                        osb2[:, ic * 128:(ic + 1) * 128], identF)
                nc.vector.tensor_copy(
                    out=oatt[b][:, ih * 4:(ih + 1) * 4, 4 * g:4 * g + 4, :],
                    in_=oTr[:, :, :, 0:16])

            # ================= tail =================
            def ln_half(srcv, stt, mvt, rstdt, half):
                # stats + rstd for chunks [half*4, half*4+4)
                for k in range(4):
                    sc = half * 4 + k
                    nc.vector.bn_stats(out=stt[:, sc, :], in_=srcv[:, sc, :])
                    nc.vector.bn_aggr(out=mvt[:, sc, :], in_=stt[:, sc, :])
                h4 = slice(half * 4, half * 4 + 4)
                sd = work.tile([128, 8, 1], dt.float32, tag="sd", name="sd",
                               bufs=2)
                nc.scalar.activation(sd[:, h4], mvt[:, h4, 1:2], AF.Sqrt,
                                     bias=eps_ap)
                nc.vector.reciprocal(out=rstdt[:, h4], in_=sd[:, h4])

            def tail(b):
                oav = oatt[b].rearrange("p a b c -> p a (b c)")  # [128,8,128]
                stt = work.tile([128, 8, 6], dt.float32, tag="stt",
                                name="stt", bufs=2)
                mv1 = work.tile([128, 8, 2], dt.float32, tag="mv1",
                                name="mv1", bufs=2)
                rstd1 = work.tile([128, 8, 1], dt.float32, tag="rstd1",
                                  name="rstd1", bufs=2)
                if KDBG and b == 0:
                    nc.sync.dma_start(out=dbg["oatt0"][:], in_=oav)
                res1 = work.tile([128, 8, 128], dt.bfloat16, tag="res1",
                                 name="res1", bufs=1)
                stt2 = work.tile([128, 8, 6], dt.float32, tag="stt2",
                                 name="stt2", bufs=2)
                mv2 = work.tile([128, 8, 2], dt.float32, tag="mv2",
                                name="mv2", bufs=2)
                rstd2 = work.tile([128, 8, 1], dt.float32, tag="rstd2",
                                  name="rstd2", bufs=2)
                att = work.tile([128, 8, 128], dt.float32, tag="att",
                                name="att", bufs=1)
                attT = work.tile([128, S], dt.bfloat16, tag="attT",
                                 name="attT", bufs=2)
                for half in range(2):
                    ln_half(oav, stt, mv1, rstd1, half)
                    for k in range(4):
                        sc = half * 4 + k
                        o1 = work.tile([128, 128], dt.bfloat16, tag="o1",
                                       name="o1", bufs=2)
                        nc.vector.tensor_scalar(o1, oav[:, sc, :],
                                                mv1[:, sc, 0:1],
                                                rstd1[:, sc, :],
                                                ALU.subtract, ALU.mult)
                        if not ln_identity:
                            nc.vector.tensor_tensor(o1, o1, lnG[:, 0, :],
                                                    op=ALU.mult)
                            nc.vector.tensor_tensor(o1, o1, lnB[:, 0, :],
                                                    op=ALU.add)
                        nc.vector.tensor_tensor(res1[:, sc, :], o1,
                                                xsrc[b][:, sc, :], op=ALU.add)
                    ln_half(res1, stt2, mv2, rstd2, half)
                    for k in range(4):
                        sc = half * 4 + k
                        nc.vector.tensor_scalar(
                            att[:, sc, :], res1[:, sc, :], mv2[:, sc, 0:1],
                            rstd2[:, sc, :], ALU.subtract, ALU.mult)
                        if not ln_identity:
                            nc.vector.tensor_tensor(att[:, sc, :],
                                                    att[:, sc, :],
                                                    lnG[:, 1, :], op=ALU.mult)
                            nc.vector.tensor_tensor(att[:, sc, :],
                                                    att[:, sc, :],
                                                    lnB[:, 1, :], op=ALU.add)
                    ps = TO()
                    psr = ps.rearrange("p (k m) -> p k m", k=4)
                    for k in range(4):
                        sc = half * 4 + k
                        nc.tensor.transpose(psr[:, k, :], att[:, sc, :],
                                            identF)
                    nc.vector.tensor_copy(
                        out=attT[:, half * 512:(half + 1) * 512],
                        in_=psr)
                if KDBG and b == 0:
                    av = att.rearrange("p a c -> p (a c)")
                    nc.sync.dma_start(out=dbg["att0"][:], in_=av)
                hrelu = work.tile([128, 4, S], dt.bfloat16, tag="hrelu",
                                  name="hrelu", bufs=1)
                ffT = work.tile([128, S], dt.float32, tag="ffT", name="ffT",
                                bufs=1)
                for sh in range(2):
                    for fc in range(4):
                        ps = TB()
                        nc.tensor.matmul(
                            ps, lhsT=ffw1T[:, fc * 128:(fc + 1) * 128],
                            rhs=attT[:, sh * 512:(sh + 1) * 512],
                            start=True, stop=True)
                        nc.vector.tensor_scalar(
                            hrelu[:, fc, sh * 512:(sh + 1) * 512], ps,
                            ffb1[:, fc:fc + 1], 0.0, ALU.add, ALU.max)
                    ps = TA() if sh == 0 else TB()
                    for fc in range(4):
                        nc.tensor.matmul(
                            ps, lhsT=ffw2T[:, fc, :],
                            rhs=hrelu[:, fc, sh * 512:(sh + 1) * 512],
                            start=(fc == 0), stop=(fc == 3))
                    nc.vector.tensor_scalar(
                        ffT[:, sh * 512:(sh + 1) * 512], ps, ffb2, None,
                        ALU.add)
                if KDBG and b == 0:
                    nc.sync.dma_start(out=dbg["ffT"][:], in_=ffT)
                l2in = work.tile([128, 8, 128], dt.float32, tag="l2in",
                                 name="l2in", bufs=1)
                stt3 = work.tile([128, 8, 6], dt.float32, tag="stt3",
                                 name="stt3", bufs=2)
                mv3 = work.tile([128, 8, 2], dt.float32, tag="mv3",
                                name="mv3", bufs=2)
                rstd3 = work.tile([128, 8, 1], dt.float32, tag="rstd3",
                                  name="rstd3", bufs=2)
                gacc = TR()
                for half in range(2):
                    ps = TR() if half == 2 else TO()
                    psr = ps.rearrange("p (k m) -> p k m", k=4)
                    for k in range(4):
                        sc = half * 4 + k
                        nc.tensor.transpose(psr[:, k, :],
                                            ffT[:, sc * 128:(sc + 1) * 128],
                                            identF)
                    for k in range(4):
                        sc = half * 4 + k
                        nc.vector.tensor_tensor(l2in[:, sc, :],
                                                att[:, sc, :], psr[:, k, :],
                                                op=ALU.add)
                    ln_half(l2in, stt3, mv3, rstd3, half)
                    if ln_identity:
                        # GAP folded with LN2: sum_i (x-m)*r
                        #   = sum_i x*r - sum_i m*r  (per output channel e)
                        for k in range(4):
                            sc = half * 4 + k
                            nc.tensor.matmul(gacc[:, 0:1],
                                             lhsT=l2in[:, sc, :],
                                             rhs=rstd3[:, sc, :],
                                             start=(sc == 0), stop=(sc == 7),
                                             skip_group_check=True)
                    else:
                        for k in range(4):
                            sc = half * 4 + k
                            l2o = work.tile([128, 128], dt.float32,
                                            tag="l2o", name="l2o", bufs=2)
                            nc.vector.tensor_scalar(
                                l2o, l2in[:, sc, :], mv3[:, sc, 0:1],
                                rstd3[:, sc, :], ALU.subtract, ALU.mult)
                            nc.vector.tensor_tensor(l2o, l2o, lnG[:, 2, :],
                                                    op=ALU.mult)
                            nc.vector.tensor_tensor(l2o, l2o, lnB[:, 2, :],
                                                    op=ALU.add)
                            nc.tensor.matmul(gacc[:, 0:1], lhsT=l2o,
                                             rhs=onesf,
                                             start=(sc == 0), stop=(sc == 7),
                                             skip_group_check=True)
                ob = work.tile([128, 1], dt.float32, tag="ob", name="ob",
                               bufs=2)
                if ln_identity:
                    # mrsum = sum_{i,sc} m*r, scaled by 1/S and broadcast
                    mrs = work.tile([128, 8, 1], dt.float32, tag="mrs",
                                    name="mrs", bufs=2)
                    mrsum = work.tile([128, 1], dt.float32, tag="mrsum",
                                      name="mrsum", bufs=2)
                    nc.vector.scalar_tensor_tensor(
                        mrs, mv3[:, :, 0:1], 1.0, rstd3, ALU.mult, ALU.mult,
                        accum_out=mrsum)
                    mr11 = TO()
                    nc.tensor.matmul(mr11[0:1, 0:1], lhsT=mrsum, rhs=invS,
                                     start=True, stop=True)
                    mr11s = work.tile([1, 1], dt.float32, tag="mr11s",
                                      name="mr11s", bufs=2)
                    nc.vector.tensor_copy(out=mr11s, in_=mr11[0:1, 0:1])
                    mrbc = TO()
                    nc.tensor.matmul(mrbc[:, 0:1], lhsT=onesRow, rhs=mr11s,
                                     start=True, stop=True)
                    mrbs = work.tile([128, 1], dt.float32, tag="mrbs",
                                     name="mrbs", bufs=2)
                    nc.vector.tensor_copy(out=mrbs, in_=mrbc[:, 0:1])
                    nc.vector.scalar_tensor_tensor(
                        ob, gacc[:, 0:1], 1.0 / S, mrbs,
                        ALU.mult, ALU.subtract)
                else:
                    nc.vector.tensor_scalar(ob, gacc[:, 0:1], 1.0 / S, None,
                                            ALU.mult)
                nc.sync.dma_start(out=yout[b, :, None], in_=ob)

            # ---------------- emission order ----------------
            stem(0)
            qkv(0)
            stem(1)
            for g in range(NG):
                for ih in range(2):
                    attn(0, g, ih)
            qkv(1)
            tail(0)
            for g in range(NG):
                for ih in range(2):
                    attn(1, g, ih)
            tail(1)

    nc.compile()
    return nc


_CACHE = {}


def _make_in_maps(inputs, host):
    x = np.asarray(inputs["x"], dtype=F32)                 # [B, S, 4]
    xpad = np.zeros((B, S + 7, C_IN), F32)
    xpad[:, 3:S + 3, :] = x
    rhs_rep = np.zeros((B, 128, C_IN, S), F32)
    for t in range(8):
        shifted = xpad[:, t:t + S, :].transpose(0, 2, 1)   # [B, 4, S]
        for cc in range(4):
            rhs_rep[:, 32 * cc + t] = shifted
    rhs_rep[:, [32 * cc + 8 for cc in range(4)]] = 1.0
    rhs_rep = np.ascontiguousarray(
        rhs_rep.reshape(B, 128, C_IN * S).astype(BF16))
    in_maps = []
    for core in range(N_CORES):
        m = {"rhs_rep": np.ascontiguousarray(
            rhs_rep[core * NB:(core + 1) * NB])}
        m.update(host)
        in_maps.append(m)
    return in_maps


def _get_nc(inputs):
    host, fidx, bidx, ln_identity = _host_prep(inputs)
    key = (ln_identity, host["fpack"].shape[1], host["bpack"].shape[1],
           KDBG, KDBG_G)
    if key not in _CACHE:
        _CACHE[key] = _build_bass(fidx, bidx, host["fpack"].shape[1],
                                  host["bpack"].shape[1], ln_identity)
    return _CACHE[key], host


def kernel(**inputs):
    inputs = {k: np.asarray(v) for k, v in inputs.items()}
    nc, host = _get_nc(inputs)
    from concourse.bass_utils import run_bass_kernel_spmd
    in_maps = _make_in_maps(inputs, host)
    res = run_bass_kernel_spmd(nc, in_maps, list(range(N_CORES)))
    if KDBG:
        kernel.dbg = res.results[0]
    outs = [res.results[c]["y"] for c in range(N_CORES)]
    return np.concatenate(outs, axis=0).astype(F32)


def build(inputs):
    inputs = {k: np.asarray(v) for k, v in inputs.items()}
    nc, host = _get_nc(inputs)
    return nc, _make_in_maps(inputs, host)
